# revision 1
# baseline (speedup 1.0000x reference)
"""LocalGainNorm Trainium2 kernel.

Math (per (b, c) lane, causal over time t):
    s_t = ALPHA * s_{t-1} + BETA * x_t^2        (s_{-1} = 0)
    g_t = sqrt(s_t + EPS)
    y_t = x_t / (g_t + EPS)

Strategy (per core; batch sharded 8 ways):
  - Layout: time on SBUF partitions, (b, c) on the free axis; 4 batch rows
    colocated per tile (free = 4*512 = 2048).
  - The causal EMA is a blocked scan done on the TensorEngine: a chunk of
    127 timesteps is one K=128 fp32 matmul with a lower-triangular decay
    matrix; the 128th rhs row is a carry slot holding the running state, and
    the output's row 127 duplicates row 126 so the next chunk's carry can be
    extracted lane-aligned (PSUM row 127 -> next q-hat row 127).
  - 1/(sqrt(s+eps)+eps) = rsqrt (ACT spline) -> one Newton step (custom DVE
    op) -> fused x*u*(1-eps*u) (custom DVE op).  Max rel err ~2e-7.
  - Engine balance: POOL does x^2, ACT does rsqrt + carry extract, DVE does
    Newton + fused output, PE does the scan matmuls; DMA via HWDGE.
"""
import sys

sys.path.insert(0, "/opt/trn_rl_repo")

import numpy as np
from contextlib import ExitStack

import concourse.bass as bass
import concourse.mybir as mybir
import concourse.tile as tile
from concourse import bacc
from concourse.bass_utils import run_bass_kernel_spmd

import concourse.dve_ops as dve_ops
from concourse.dve_ops import DveOp, OPS, CUSTOM_DVE_SPECS, get_dve_sub_opcode
from concourse.dve_spec import Spec, Src0, Src1, C0, C1, C2, One, sq, lower, _has_src1
from concourse.dve_uop import DveOpSpec

F32 = mybir.dt.float32
AF = mybir.ActivationFunctionType

B, T, C = 64, 2048, 512
ALPHA, BETA, EPS = 0.98, 1.0 - 0.98, 1e-6
NCORES = 8
BPC = B // NCORES        # batch rows per core (8)
NB = 4                   # batch rows colocated per tile group
NGROUPS = BPC // NB      # 2
LCH = 127                # timesteps per full chunk (row 127 = carry slot)
NFULL = T // LCH         # 16
TAIL = T - NFULL * LCH   # 16
NCHUNK = NFULL + (1 if TAIL else 0)
FREE = NB * C            # 2048


def _register_dve_op(name, spec):
    if any(op.name == name for op in OPS):
        return next(op for op in OPS if op.name == name)
    op = DveOp(name, spec, subdim=False, uops_sha={})
    OPS.append(op)
    CUSTOM_DVE_SPECS[name] = spec
    dve_ops._SUB_OPCODE_FOR_NAME[name] = dve_ops._CUSTOM_DVE_ROW_BASE + len(OPS) - 1
    for ver in ("v3", "v4"):
        ds = DveOpSpec(
            name=name,
            opcode=get_dve_sub_opcode(name),
            uops=lower(spec, ver=ver),
            rd1_en=_has_src1(spec),
        )
        op.uops_sha[ver] = ds.sha(ver)
    return op


# w1 = w0 * (imm2 - s1*((s + s0) * w0^2));  Src0 = s (PSUM ok), Src1 = w0
NR_OP = _register_dve_op(
    "EMA_RSQRT_NR",
    Spec(
        body=Src1 * (C2 - C1 * ((Src0 + C0) * sq(Src1))),
        reference=lambda in0, in1, s0, s1, imm2: in1
        * (imm2 - s1 * ((in0 + s0) * in1 * in1)),
    ),
)

# out = x * (u * (1 - s0*u));  Src0 = x, Src1 = u
OUT_OP = _register_dve_op(
    "EMA_OUT_FUSED",
    Spec(
        body=Src0 * (Src1 * (One - C0 * Src1)),
        reference=lambda in0, in1, s0, s1, imm2: in0 * (in1 * (1.0 - s0 * in1)),
    ),
)


def _build_amats():
    # Full chunk: A[t, i] over t in 0..127 (outputs), i in 0..127 (rhs rows).
    # Rows 0..126: s(t0+t); row 127 duplicates row 126 (carry emission).
    # Cols 0..126: data coeffs BETA*ALPHA^(t-i); col 127: carry coeff ALPHA^(t+1).
    A = np.zeros((128, 128), np.float64)
    for t in range(LCH):
        for i in range(t + 1):
            A[t, i] = BETA * ALPHA ** (t - i)
        A[t, 127] = ALPHA ** (t + 1)
    A[127, :] = A[126, :]
    # Tail chunk: outputs t in 0..TAIL-1; rhs rows 0..TAIL-1 data, 127 carry,
    # rows TAIL..126 are zero columns (rhs there is memset to 0 anyway).
    At = np.zeros((TAIL, 128), np.float64)
    for t in range(TAIL):
        for i in range(t + 1):
            At[t, i] = BETA * ALPHA ** (t - i)
        At[t, 127] = ALPHA ** (t + 1)
    return (
        np.ascontiguousarray(A.T).astype(np.float32),      # lhsT (128, 128)
        np.ascontiguousarray(At.T).astype(np.float32),     # lhsT tail (128, TAIL)
    )


_BUILT = {}


def _act_raw(nc, out_ap, in_ap, func, bias_val, scale=1.0):
    """Raw-emit an activation (bypasses the bass Rsqrt precision gate)."""
    eng = nc.scalar
    bias_ap = nc.const_aps.scalar_like(bias_val, in_ap)
    ins_l = [
        eng.lower_ap(in_ap),
        eng.lower_ap(bias_ap),
        mybir.ImmediateValue(dtype=F32, value=scale),
        mybir.ImmediateValue(dtype=F32, value=0.0),
    ]
    return eng.add_instruction(
        mybir.InstActivation(
            name=nc.get_next_instruction_name(),
            func=func,
            ins=ins_l,
            outs=[eng.lower_ap(out_ap)],
        )
    )


def _build(repeat=1, variant="full"):
    key = (repeat, variant)
    if key in _BUILT:
        return _BUILT[key]

    nc = bacc.Bacc(
        "TRN2",
        target_bir_lowering=False,
        debug=False,
        enable_asserts=False,
        num_devices=NCORES,
    )
    # eps const AP for the rsqrt bias
    eps_t = nc.alloc_sbuf_tensor("const-f32-eps", [128, 1], F32)
    nc.gpsimd.memset(eps_t.ap(), EPS)
    nc.const_aps.aps[(F32, EPS)] = eps_t.ap()
    nc.all_engine_barrier()

    x_in = nc.dram_tensor("x", [BPC, T, C], F32, kind="ExternalInput")
    a_in = nc.dram_tensor("amat", [128, 128], F32, kind="ExternalInput")
    at_in = nc.dram_tensor("amat_tail", [128, TAIL], F32, kind="ExternalInput")
    y_out = nc.dram_tensor("y", [BPC, T, C], F32, kind="ExternalOutput")

    if variant.startswith("dma_pbmc"):
        # per-b multi-chunk: one dma_start per (b, half, direction) covering
        # KCH chunks (contiguous DRAM region, 3D AP, 2KB runs, <=1016 descs)
        KCH = int(variant.split("-")[1]) if "-" in variant else 8
        span = KCH * LCH
        nhalf = NFULL // KCH
        with tile.TileContext(nc) as tc:
            with ExitStack() as ctx:
                wpool2 = ctx.enter_context(tc.tile_pool(name="xw", bufs=3))
                for rep in range(repeat):
                    for bb in range(BPC):
                        for h in range(nhalf):
                            xw = wpool2.tile([LCH, KCH * C], F32, tag="xw", name=f"xw_{rep}_{bb}_{h}")
                            sl = slice(h * span, (h + 1) * span)
                            src = x_in[bb, sl, :].rearrange("(k p) c -> p k c", p=LCH)
                            x3 = xw[:].rearrange("p (k c) -> p k c", c=C)
                            nc.gpsimd.dma_start(x3, src)
                            dst = y_out[bb, sl, :].rearrange("(k p) c -> p k c", p=LCH)
                            nc.gpsimd.dma_start(dst, x3)
        nc.compile()
        _BUILT[key] = (nc, _build_amats())
        return _BUILT[key]

    if variant.startswith("dma_flat"):
        # pure bandwidth test: big contiguous transfers, no compute
        nparts, fsz = 128, (512 if variant == "dma_flat512" else 4096)
        ntiles = (BPC * T * C) // (nparts * fsz)  # 16 x 2MB per core
        xf = x_in.rearrange("b t c -> (b t c)").rearrange(
            "(n p f) -> n p f", p=nparts, f=fsz
        )
        yf = y_out.rearrange("b t c -> (b t c)").rearrange(
            "(n p f) -> n p f", p=nparts, f=fsz
        )
        with tile.TileContext(nc) as tc:
            with ExitStack() as ctx:
                fpool = ctx.enter_context(tc.tile_pool(name="flat", bufs=6))
                for rep in range(repeat):
                    for n in range(ntiles):
                        ft = fpool.tile([nparts, fsz], F32, tag="flat", name=f"f_{rep}_{n}")
                        nc.sync.dma_start(ft[:], xf[n])
                        nc.sync.dma_start(yf[n], ft[:])
        nc.compile()
        _BUILT[key] = (nc, _build_amats())
        return _BUILT[key]

    with tile.TileContext(nc) as tc:
        with ExitStack() as ctx:
            cpool = ctx.enter_context(tc.tile_pool(name="consts", bufs=1))
            xpool = ctx.enter_context(tc.tile_pool(name="x", bufs=7))
            qpool = ctx.enter_context(tc.tile_pool(name="qhat", bufs=6))
            wpool = ctx.enter_context(tc.tile_pool(name="w0", bufs=3))
            upool = ctx.enter_context(tc.tile_pool(name="w1", bufs=3))
            opool = ctx.enter_context(tc.tile_pool(name="outs", bufs=4))
            ppool = ctx.enter_context(
                tc.tile_pool(name="psum", bufs=2, space="PSUM")
            )

            a_t = cpool.tile([128, 128], F32)
            nc.sync.dma_start(a_t[:], a_in[:])
            at_t = cpool.tile([128, TAIL], F32)
            nc.sync.dma_start(at_t[:], at_in[:])

            # q-hat tiles are created lazily; chunk k's extract writes into
            # chunk k+1's tile.
            qhat = {}

            def get_qhat(rep, g, k):
                if (rep, g, k) not in qhat:
                    t = qpool.tile([128, FREE], F32, tag="qhat", name=f"qh_{rep}_{g}_{k}")
                    qhat[(rep, g, k)] = t
                    if k == NFULL or k == 0:
                        # chunk-0: carry row must start 0; tail: rows
                        # TAIL..126 must be zero. Memset at creation so it
                        # precedes the carry extract / squares (WAW order).
                        nc.vector.memset(t[:, :], 0.0)
                return qhat[(rep, g, k)]

            for rep in range(repeat):
              for k in range(NCHUNK):
                is_tail = k == NFULL
                L = TAIL if is_tail else LCH
                t0 = k * LCH
                for g in range(NGROUPS):
                    b0 = g * NB
                    q_t = get_qhat(rep, g, k)

                    x_t = xpool.tile([L, FREE], F32, tag="x", name=f"xt_{rep}_{g}_{k}")
                    if variant == "dma_only_gp":
                        src = x_in[b0 : b0 + NB, t0 : t0 + L, :].rearrange(
                            "b t c -> t b c"
                        )
                        x3 = x_t[:].rearrange("t (b c) -> t b c", b=NB)
                        nc.gpsimd.dma_start(x3, src)
                    elif "perb" in variant:
                        # one contiguous-DRAM transfer per batch row
                        for bb in range(NB):
                            eng = nc.gpsimd if variant.endswith("gp") else (
                                nc.scalar if (variant.endswith("2ring") and bb % 2) else nc.sync)
                            eng.dma_start(
                                x_t[:, bb * C : (bb + 1) * C],
                                x_in[b0 + bb, t0 : t0 + L, :],
                            )
                    else:
                        # full kernel: per-b contiguous 2D transfers; SWDGE
                        # descgen is the bottleneck, so route 1/8 of the
                        # transfers through the sync HWDGE ring
                        for bb in range(NB):
                            eng = nc.sync if (g == 1 and bb == 3) else nc.gpsimd
                            eng.dma_start(
                                x_t[:, bb * C : (bb + 1) * C],
                                x_in[b0 + bb, t0 : t0 + L, :],
                            )

                    if variant.startswith("dma_only"):
                        if "perb" in variant:
                            for bb in range(NB):
                                eng = nc.gpsimd if variant.endswith("gp") else (
                                    nc.scalar if (variant.endswith("2ring") and bb % 2) else nc.sync)
                                eng.dma_start(
                                    y_out[b0 + bb, t0 : t0 + L, :],
                                    x_t[:, bb * C : (bb + 1) * C],
                                )
                        else:
                            dst = y_out[b0 : b0 + NB, t0 : t0 + L, :].rearrange(
                                "b t c -> t b c"
                            )
                            x3b = x_t[:].rearrange("t (b c) -> t b c", b=NB)
                            eng = nc.gpsimd if variant.endswith("gp") else nc.sync
                            eng.dma_start(dst, x3b)
                        continue

                    # squares into rows 0..L-1 (ACT for group 0, DVE for
                    # group 1 — POOL is reserved for DMA descriptor gen)
                    if variant == "sq_dve" or (variant == "full" and g == 1):
                        nc.vector.tensor_mul(q_t[0:L, :], x_t[:], x_t[:])
                    elif variant == "sq_act" or variant == "full":
                        nc.scalar.activation(q_t[0:L, :], x_t[:], AF.Square)
                    else:
                        nc.gpsimd.tensor_mul(q_t[0:L, :], x_t[:], x_t[:])

                    # the scan matmuls (one PSUM bank per 512-wide slice)
                    ps = ppool.tile([128, FREE], F32, tag="psum", name=f"ps_{rep}_{g}_{k}")
                    lhsT = at_t if is_tail else a_t
                    m_rows = TAIL if is_tail else 128
                    for j in range(NB):
                        nc.tensor.matmul(
                            ps[0:m_rows, j * C : (j + 1) * C],
                            lhsT[:],
                            q_t[:, j * C : (j + 1) * C],
                            start=True,
                            stop=True,
                        )

                    if not is_tail:
                        # Carry to next chunk. Engine APs must start on a
                        # 32-aligned partition, so copy the whole [96:128)
                        # quadrant slice (lane-preserving); rows 96..126 are
                        # overwritten by the next chunk's squares (WAW order
                        # via emission), row 127 is the carry. Split in two
                        # free-halves so each half can start as soon as its
                        # two PSUM banks are written.
                        q_next = get_qhat(rep, g, k + 1)
                        nc.scalar.copy(q_next[96:128, :], ps[96:128, :])

                    # w0 = rsqrt(s + eps)
                    w0_t = wpool.tile([128, FREE], F32, tag="w0", name=f"w0_{rep}_{g}_{k}")
                    _act_raw(nc, w0_t[0:L, :], ps[0:L, :], AF.Rsqrt, EPS)

                    # Newton refinement
                    if variant == "no_nr":
                        w1_t = w0_t
                    else:
                        w1_t = upool.tile([128, FREE], F32, tag="w1", name=f"w1_{rep}_{g}_{k}")
                        nc.vector._custom_dve(
                            NR_OP,
                            out=w1_t[0:L, :],
                            in0=ps[0:L, :],
                            in1=w0_t[0:L, :],
                            s0=EPS,
                            s1=0.5,
                            imm2=1.5,
                        )

                    # fused normalize
                    o_t = opool.tile([L, FREE], F32, tag="outs", name=f"ot_{rep}_{g}_{k}")
                    nc.vector._custom_dve(
                        OUT_OP,
                        out=o_t[:],
                        in0=x_t[:],
                        in1=w1_t[0:L, :],
                        s0=EPS,
                    )

                    for bb in range(NB):
                        if g == 0 and bb == 3:
                            eng = nc.sync
                        elif variant == "act_dma" and g == 1 and bb == 3:
                            eng = nc.scalar
                        else:
                            eng = nc.gpsimd
                        eng.dma_start(
                            y_out[b0 + bb, t0 : t0 + L, :],
                            o_t[:, bb * C : (bb + 1) * C],
                        )

    nc.compile()
    _BUILT[key] = (nc, _build_amats())
    return _BUILT[key]


def kernel(batch_x: np.ndarray) -> np.ndarray:
    nc, (a_np, at_np) = _build()
    batch_x = np.ascontiguousarray(batch_x, dtype=np.float32)
    in_maps = [
        {
            "x": batch_x[i * BPC : (i + 1) * BPC],
            "amat": a_np,
            "amat_tail": at_np,
        }
        for i in range(NCORES)
    ]
    res = run_bass_kernel_spmd(nc, in_maps, list(range(NCORES)))
    out = np.concatenate([res.results[i]["y"] for i in range(NCORES)], axis=0)
    return out.astype(np.float32, copy=False)


if __name__ == "__main__":
    rng = np.random.default_rng(0)
    x = rng.normal(size=(B, T, C)).astype(np.float32)
    y = kernel(x)
    print("out", y.shape, y.dtype)

